# revision 20
# baseline (speedup 1.0000x reference)
"""BinaryLinear TRN2 kernel.

Computes out = inputs @ (sign(W) * scale).T + bias where
  sign(w) = +1 for w >= 0 else -1
  scale[o] = max(mean_i |W[o, i]|, 1e-6)

Problem shapes (hardcoded): inputs [8192, 4096] f32, weight [4096, 4096] f32,
bias [4096] f32 -> out [8192, 4096] f32.

Distribution: data-parallel over tokens. Each of the 8 cores gets a
[1024, 4096] slice of `inputs` and the full weight/bias, and produces a
[1024, 4096] slice of the output. No collectives; host concatenates.

Per-core algorithm (Tile framework):
  - X^T: cast X shard to bf16, PE-transpose 128x128 tiles, keep the whole
    [4096 x 1024] X^T resident in SBUF as [128, 32, 1024] bf16.
  - Weights: stream W by 128-row output tiles; ACT computes sign() into bf16
    (exact +-1, so the matmul sees the binary matrix exactly); DVE computes
    row sums of |W| (scale); PE transposes sign tiles into S^T [128, 32, 512]
    bf16 per 512-column output chunk.
  - Matmul: psum[t128, o512] accumulates over the 32 k-tiles,
    lhsT = X^T tile (stationary), rhs = S^T tile (moving).
  - Evict: out = psum * scale[o] + bias[o] (both broadcast along partitions
    via a stride-0 DMA), DMA to DRAM.

Only X's bf16 rounding introduces error (~1e-3 relative); sign matrix is
exact, accumulation in fp32 PSUM, scale/bias applied in fp32.
"""

import os
import sys

import numpy as np

sys.path.insert(0, "/opt/trn_rl_repo")

import concourse.bass as bass
import concourse.mybir as mybir
from concourse import bacc
import concourse.tile as tile
from concourse.masks import make_identity


def _ensure_ntff_hook():
    """The agent image's `antenv` lacks `axon_hooks`, which
    run_bass_kernel_spmd imports when trace=True (for HW exec timing).
    Provide the module and install the standard ctypes-based hook.
    Harmless when tracing is off (the import never fires)."""
    import types

    try:
        import antenv.axon_hooks  # noqa: F401
        return
    except ImportError:
        pass
    try:
        import antenv
    except ImportError:
        return
    mod = types.ModuleType("antenv.axon_hooks")
    state = {"hook": None}
    mod.set_axon_ntff_profile_hook = lambda h: state.update(hook=h)
    mod.get_axon_ntff_profile_hook = lambda: state["hook"]
    sys.modules["antenv.axon_hooks"] = mod
    antenv.axon_hooks = mod
    try:
        from trn_agent_boot.trn_boot import _ntff_profile_via_ctypes

        hook = _ntff_profile_via_ctypes("/opt/axon/libaxon_pjrt.so")
        if hook is not None:
            mod.set_axon_ntff_profile_hook(hook)
    except Exception:
        pass


_ensure_ntff_hook()

F32 = mybir.dt.float32
BF16 = mybir.dt.bfloat16

TOKENS = 8192
IN_FEATURES = 4096
OUT_FEATURES = 4096
N_CORES = 8


def build_nc(t_core, in_f, out_f, ich=1024, och=512):
    """Build the per-core Bass module. All cores run the identical program."""
    P = 128
    assert t_core % P == 0 and in_f % ich == 0 and out_f % och == 0
    assert ich % P == 0 and och % P == 0
    t_tiles = t_core // P          # token tiles per core
    i_tiles = in_f // P            # contraction (k) tiles
    i_chunks = in_f // ich         # staging chunks along k
    isub = ich // P                # k tiles per staging chunk
    oc_chunks = out_f // och       # output column chunks
    o_tiles = och // P             # 128-row W tiles per output chunk

    # Bacc (not plain Bass): its compile pipeline runs
    # generate_event_semaphores, which legalizes multi-wait DMAs
    # (walrus allows at most one sync wait per instruction).
    nc = bacc.Bacc()
    x_dram = nc.dram_tensor("x", [t_core, in_f], F32, kind="ExternalInput")
    w_dram = nc.dram_tensor("w", [out_f, in_f], F32, kind="ExternalInput")
    b_dram = nc.dram_tensor("b", [out_f], F32, kind="ExternalInput")
    out_dram = nc.dram_tensor("out", [t_core, out_f], F32, kind="ExternalOutput")

    with tile.TileContext(nc) as tc:
        sch = min(2048, in_f)      # staging chunk (8KB DMA packets)
        assert in_f % sch == 0 and sch % ich == 0
        s_chunks = in_f // sch
        ssub = sch // P            # k tiles per staging chunk
        tsub = ich // P            # transposes per psum bank tile
        with (
            tc.tile_pool(name="const", bufs=1) as const,
            tc.tile_pool(name="resident", bufs=1) as resident,
            tc.tile_pool(name="stage32", bufs=4) as stage32,
            tc.tile_pool(name="stage16", bufs=6) as stage16,
            tc.tile_pool(name="st", bufs=2) as st_pool,
            tc.tile_pool(name="small", bufs=4) as small,
            tc.tile_pool(name="scbc", bufs=2) as scbc,
            tc.tile_pool(name="outsb", bufs=4) as outsb,
            tc.tile_pool(name="psum_tr", bufs=4, space="PSUM") as psum_tr,
            tc.tile_pool(name="psum_mm", bufs=4, space="PSUM") as psum_mm,
            tc.tile_pool(name="dram", bufs=1, space="DRAM") as dram_pool,
        ):
            ident = const.tile([P, P], BF16)
            make_identity(nc, ident)

            # tiny positive bias so Sign(0 + tiny) = +1, matching the
            # reference's where(w >= 0, 1, -1)
            signbias = const.tile([P, 1], F32)
            nc.vector.memset(signbias[:], 1e-30)

            # per-row scale scratch in DRAM (written column-major by o-tile,
            # read back with a partition-broadcast AP)
            scale_dram = dram_pool.tile([out_f], F32)
            # [o] viewed as [p, o_tile] so sbuf [128, g] cols map to o = g*128+p
            scale_pm = scale_dram[:].rearrange("(g p) -> p g", p=P)

            def transpose_chunk(src16, dst, ktile0):
                """PE-transpose a [128, sch] bf16 natural chunk into
                dst[:, ktile0+j, col0:col0+128] via 1-bank psum staging."""
                for h in range(sch // ich):
                    ps = psum_tr.tile([P, ich], BF16, tag="trps")
                    for j in range(tsub):
                        nc.tensor.transpose(
                            ps[:, j * P:(j + 1) * P],
                            src16[:, h * ich + j * P:h * ich + (j + 1) * P],
                            ident[:],
                        )
                    k0 = ktile0 + h * tsub
                    nc.vector.tensor_copy(
                        dst[:, k0:k0 + tsub, :],
                        ps[:].rearrange("p (s q) -> p s q", q=P),
                    )

            # ---- Phase 1: build resident X^T (bf16) ----
            xt = resident.tile([P, i_tiles, t_core], BF16)
            for t in range(t_tiles):
                for c in range(s_chunks):
                    xs = stage32.tile([P, sch], F32, tag="stage")
                    # startup is aggregate-DMA-bound: rings carry W + half of
                    # X, the otherwise-idle software queue carries the rest
                    eng = nc.sync if c % 2 == 0 else nc.gpsimd
                    eng.dma_start(
                        xs[:], x_dram[t * P:(t + 1) * P, c * sch:(c + 1) * sch]
                    )
                    xb = stage16.tile([P, sch], BF16, tag="bf16stage")
                    nc.scalar.activation(
                        xb[:], xs[:], mybir.ActivationFunctionType.Copy
                    )
                    transpose_chunk(
                        xb, xt[:, :, t * P:(t + 1) * P], c * ssub
                    )
                    # HAM warm-up: transpose-mode ops don't count as PE-busy,
                    # so phase 1 would run at the cold 1.2 GHz clock and the
                    # first real matmuls would start cold. A tiny matmul tied
                    # to each chunk's data keeps the activity monitor busy
                    # across phase 1 (result is discarded).
                    if t * s_chunks + c < 16:
                        warm = psum_mm.tile([P, och], F32, tag="mmps")
                        nc.tensor.matmul(
                            warm[:, :P], ident[:], xb[:, :P],
                            start=True, stop=True,
                        )

            # ---- Phase 2: per output-column chunk ----
            def build_st(oc):
                """Stream W rows for one 512-col output chunk: sign -> S^T
                (PE transpose), plus the per-row scale with DRAM round-trip
                into a partition-broadcast row, and the bias row chunk."""
                st = st_pool.tile([P, i_tiles, och], BF16, tag="st")
                scale_cols = small.tile([P, o_tiles], F32, tag="scale_cols")
                for ot in range(o_tiles):
                    o_row = (oc * o_tiles + ot) * P
                    red = small.tile([P, s_chunks], F32, tag="red")
                    for c in range(s_chunks):
                        ws = stage32.tile([P, sch], F32, tag="stage")
                        nc.scalar.dma_start(
                            ws[:], w_dram[o_row:o_row + P, c * sch:(c + 1) * sch]
                        )
                        sn = stage16.tile([P, sch], BF16, tag="bf16stage")
                        nc.scalar.activation(
                            sn[:], ws[:], mybir.ActivationFunctionType.Sign,
                            bias=signbias[:],
                        )
                        nc.vector.tensor_reduce(
                            red[:, c:c + 1], ws[:],
                            axis=mybir.AxisListType.X, op=mybir.AluOpType.add,
                            apply_absolute_value=True,
                        )
                        transpose_chunk(
                            sn, st[:, :, ot * P:(ot + 1) * P], c * ssub
                        )
                    redt = small.tile([P, 1], F32, tag="redt")
                    nc.vector.tensor_reduce(
                        redt[:], red[:],
                        axis=mybir.AxisListType.X, op=mybir.AluOpType.add,
                    )
                    nc.vector.tensor_scalar(
                        scale_cols[:, ot:ot + 1], redt[:],
                        1.0 / in_f, 1e-6,
                        op0=mybir.AluOpType.mult, op1=mybir.AluOpType.max,
                    )
                nc.gpsimd.dma_start(
                    scale_pm[:, oc * o_tiles:(oc + 1) * o_tiles], scale_cols[:]
                )
                sc_bc = scbc.tile([P, och], F32, tag="scbc")
                sc_slice = scale_dram[oc * och:(oc + 1) * och]
                nc.sync.dma_start(
                    sc_bc[:],
                    bass.AP(tensor=sc_slice.tensor, offset=sc_slice.offset,
                            ap=[[0, P]] + list(sc_slice.ap)),
                )
                bias_bc = scbc.tile([P, och], F32, tag="biasbc")
                b_slice = b_dram[oc * och:(oc + 1) * och]
                nc.sync.dma_start(
                    bias_bc[:],
                    bass.AP(tensor=b_slice.tensor, offset=b_slice.offset,
                            ap=[[0, P]] + list(b_slice.ap)),
                )
                return st, sc_bc, bias_bc

            def mm_block(oc, t, st, sc_bc, bias_bc):
                pm = psum_mm.tile([P, och], F32, tag="mmps")
                for i in range(i_tiles):
                    nc.tensor.matmul(
                        pm[:],
                        xt[:, i, t * P:(t + 1) * P],
                        st[:, i, :],
                        start=(i == 0), stop=(i == i_tiles - 1),
                    )
                ob = outsb.tile([P, och], F32, tag="ob")
                nc.vector.tensor_mul(out=ob[:], in0=pm[:], in1=sc_bc[:])
                nc.vector.tensor_add(out=ob[:], in0=ob[:], in1=bias_bc[:])
                # 2KB-packet output stream rides the SW queue (29 GB/s
                # needed vs ~115 available), freeing ring descriptor slots
                nc.gpsimd.dma_start(
                    out_dram[t * P:(t + 1) * P, oc * och:(oc + 1) * och],
                    ob[:],
                )

            for oc in range(oc_chunks):
                ctx = build_st(oc)
                for t in range(t_tiles):
                    mm_block(oc, t, *ctx)

    nc.finalize()
    return nc


_CACHE = {}


def kernel(inputs, weight, bias):
    from concourse.bass_utils import run_bass_kernel_spmd

    x = np.ascontiguousarray(np.asarray(inputs, dtype=np.float32))
    w = np.ascontiguousarray(np.asarray(weight, dtype=np.float32))
    b = np.ascontiguousarray(np.asarray(bias, dtype=np.float32))
    assert x.shape == (TOKENS, IN_FEATURES)
    assert w.shape == (OUT_FEATURES, IN_FEATURES)
    assert b.shape == (OUT_FEATURES,)

    if "nc" not in _CACHE:
        _CACHE["nc"] = build_nc(TOKENS // N_CORES, IN_FEATURES, OUT_FEATURES)
    nc = _CACHE["nc"]

    shards = np.split(x, N_CORES, axis=0)
    in_maps = [{"x": shards[c], "w": w, "b": b} for c in range(N_CORES)]
    trace = bool(os.environ.get("BASS_TRACE"))
    res = run_bass_kernel_spmd(nc, in_maps, list(range(N_CORES)), trace=trace)
    if trace:
        _CACHE["last_result"] = res
        if res.exec_time_ns is not None:
            print(f"HW exec time: {res.exec_time_ns} ns")

    return np.concatenate([res.results[c]["out"] for c in range(N_CORES)], axis=0)


# revision 21
# speedup vs baseline: 1.0606x; 1.0606x over previous
"""BinaryLinear TRN2 kernel.

Computes out = inputs @ (sign(W) * scale).T + bias where
  sign(w) = +1 for w >= 0 else -1
  scale[o] = max(mean_i |W[o, i]|, 1e-6)

Problem shapes (hardcoded): inputs [8192, 4096] f32, weight [4096, 4096] f32,
bias [4096] f32 -> out [8192, 4096] f32.

Distribution: data-parallel over tokens. Each of the 8 cores gets a
[1024, 4096] slice of `inputs` and the full weight/bias, and produces a
[1024, 4096] slice of the output. No collectives; host concatenates.

Per-core algorithm (Tile framework):
  - X^T: cast X shard to bf16, PE-transpose 128x128 tiles, keep the whole
    [4096 x 1024] X^T resident in SBUF as [128, 32, 1024] bf16.
  - Weights: stream W by 128-row output tiles; ACT computes sign() into bf16
    (exact +-1, so the matmul sees the binary matrix exactly); DVE computes
    row sums of |W| (scale); PE transposes sign tiles into S^T [128, 32, 512]
    bf16 per 512-column output chunk.
  - Matmul: psum[t128, o512] accumulates over the 32 k-tiles,
    lhsT = X^T tile (stationary), rhs = S^T tile (moving).
  - Evict: out = psum * scale[o] + bias[o] (both broadcast along partitions
    via a stride-0 DMA), DMA to DRAM.

Only X's bf16 rounding introduces error (~1e-3 relative); sign matrix is
exact, accumulation in fp32 PSUM, scale/bias applied in fp32.
"""

import os
import sys

import numpy as np

sys.path.insert(0, "/opt/trn_rl_repo")

import concourse.bass as bass
import concourse.mybir as mybir
from concourse import bacc
import concourse.tile as tile
from concourse.masks import make_identity


def _ensure_ntff_hook():
    """The agent image's `antenv` lacks `axon_hooks`, which
    run_bass_kernel_spmd imports when trace=True (for HW exec timing).
    Provide the module and install the standard ctypes-based hook.
    Harmless when tracing is off (the import never fires)."""
    import types

    try:
        import antenv.axon_hooks  # noqa: F401
        return
    except ImportError:
        pass
    try:
        import antenv
    except ImportError:
        return
    mod = types.ModuleType("antenv.axon_hooks")
    state = {"hook": None}
    mod.set_axon_ntff_profile_hook = lambda h: state.update(hook=h)
    mod.get_axon_ntff_profile_hook = lambda: state["hook"]
    sys.modules["antenv.axon_hooks"] = mod
    antenv.axon_hooks = mod
    try:
        from trn_agent_boot.trn_boot import _ntff_profile_via_ctypes

        hook = _ntff_profile_via_ctypes("/opt/axon/libaxon_pjrt.so")
        if hook is not None:
            mod.set_axon_ntff_profile_hook(hook)
    except Exception:
        pass


_ensure_ntff_hook()

F32 = mybir.dt.float32
BF16 = mybir.dt.bfloat16

TOKENS = 8192
IN_FEATURES = 4096
OUT_FEATURES = 4096
N_CORES = 8


def build_nc(t_core, in_f, out_f, ich=1024, och=512):
    """Build the per-core Bass module. All cores run the identical program."""
    P = 128
    assert t_core % P == 0 and in_f % ich == 0 and out_f % och == 0
    assert ich % P == 0 and och % P == 0
    t_tiles = t_core // P          # token tiles per core
    i_tiles = in_f // P            # contraction (k) tiles
    i_chunks = in_f // ich         # staging chunks along k
    isub = ich // P                # k tiles per staging chunk
    oc_chunks = out_f // och       # output column chunks
    o_tiles = och // P             # 128-row W tiles per output chunk

    # Bacc (not plain Bass): its compile pipeline runs
    # generate_event_semaphores, which legalizes multi-wait DMAs
    # (walrus allows at most one sync wait per instruction).
    nc = bacc.Bacc()
    x_dram = nc.dram_tensor("x", [t_core, in_f], F32, kind="ExternalInput")
    w_dram = nc.dram_tensor("w", [out_f, in_f], F32, kind="ExternalInput")
    b_dram = nc.dram_tensor("b", [out_f], F32, kind="ExternalInput")
    out_dram = nc.dram_tensor("out", [t_core, out_f], F32, kind="ExternalOutput")

    with tile.TileContext(nc) as tc:
        sch = min(2048, in_f)      # staging chunk (8KB DMA packets)
        assert in_f % sch == 0 and sch % ich == 0
        s_chunks = in_f // sch
        ssub = sch // P            # k tiles per staging chunk
        tsub = ich // P            # transposes per psum bank tile
        with (
            tc.tile_pool(name="const", bufs=1) as const,
            tc.tile_pool(name="resident", bufs=1) as resident,
            tc.tile_pool(name="stage32", bufs=4) as stage32,
            tc.tile_pool(name="stage16", bufs=6) as stage16,
            tc.tile_pool(name="st", bufs=2) as st_pool,
            tc.tile_pool(name="small", bufs=4) as small,
            tc.tile_pool(name="scbc", bufs=2) as scbc,
            tc.tile_pool(name="outsb", bufs=4) as outsb,
            tc.tile_pool(name="psum_tr", bufs=4, space="PSUM") as psum_tr,
            tc.tile_pool(name="psum_mm", bufs=4, space="PSUM") as psum_mm,
            tc.tile_pool(name="dram", bufs=1, space="DRAM") as dram_pool,
        ):
            ident = const.tile([P, P], BF16)
            make_identity(nc, ident)

            # tiny positive bias so Sign(0 + tiny) = +1, matching the
            # reference's where(w >= 0, 1, -1)
            signbias = const.tile([P, 1], F32)
            nc.vector.memset(signbias[:], 1e-30)

            # per-row scale scratch in DRAM (written column-major by o-tile,
            # read back with a partition-broadcast AP)
            scale_dram = dram_pool.tile([out_f], F32)
            # [o] viewed as [p, o_tile] so sbuf [128, g] cols map to o = g*128+p
            scale_pm = scale_dram[:].rearrange("(g p) -> p g", p=P)

            def transpose_chunk(src16, dst, ktile0):
                """PE-transpose a [128, sch] bf16 natural chunk into
                dst[:, ktile0+j, col0:col0+128] via 1-bank psum staging."""
                for h in range(sch // ich):
                    ps = psum_tr.tile([P, ich], BF16, tag="trps")
                    for j in range(tsub):
                        nc.tensor.transpose(
                            ps[:, j * P:(j + 1) * P],
                            src16[:, h * ich + j * P:h * ich + (j + 1) * P],
                            ident[:],
                        )
                    k0 = ktile0 + h * tsub
                    nc.vector.tensor_copy(
                        dst[:, k0:k0 + tsub, :],
                        ps[:].rearrange("p (s q) -> p s q", q=P),
                    )

            # ---- Phase 1: build resident X^T (bf16) ----
            xt = resident.tile([P, i_tiles, t_core], BF16)
            for t in range(t_tiles):
                for c in range(s_chunks):
                    xs = stage32.tile([P, sch], F32, tag="stage")
                    nc.sync.dma_start(
                        xs[:], x_dram[t * P:(t + 1) * P, c * sch:(c + 1) * sch]
                    )
                    xb = stage16.tile([P, sch], BF16, tag="bf16stage")
                    nc.scalar.activation(
                        xb[:], xs[:], mybir.ActivationFunctionType.Copy
                    )
                    transpose_chunk(
                        xb, xt[:, :, t * P:(t + 1) * P], c * ssub
                    )
                    # HAM warm-up: transpose-mode ops don't count as PE-busy,
                    # so phase 1 would run at the cold 1.2 GHz clock and the
                    # first real matmuls would start cold. A tiny matmul tied
                    # to each chunk's data keeps the activity monitor busy
                    # across phase 1 (result is discarded).
                    if t * s_chunks + c < 16:
                        warm = psum_mm.tile([P, och], F32, tag="mmps")
                        nc.tensor.matmul(
                            warm[:, :P], ident[:], xb[:, :P],
                            start=True, stop=True,
                        )

            # ---- Phase 2: per output-column chunk ----
            def build_st(oc):
                """Stream W rows for one 512-col output chunk: sign -> S^T
                (PE transpose), plus the per-row scale with DRAM round-trip
                into a partition-broadcast row, and the bias row chunk."""
                st = st_pool.tile([P, i_tiles, och], BF16, tag="st")
                scale_cols = small.tile([P, o_tiles], F32, tag="scale_cols")
                for ot in range(o_tiles):
                    o_row = (oc * o_tiles + ot) * P
                    red = small.tile([P, s_chunks], F32, tag="red")
                    for c in range(s_chunks):
                        ws = stage32.tile([P, sch], F32, tag="stage")
                        nc.scalar.dma_start(
                            ws[:], w_dram[o_row:o_row + P, c * sch:(c + 1) * sch]
                        )
                        sn = stage16.tile([P, sch], BF16, tag="bf16stage")
                        nc.scalar.activation(
                            sn[:], ws[:], mybir.ActivationFunctionType.Sign,
                            bias=signbias[:],
                        )
                        nc.vector.tensor_reduce(
                            red[:, c:c + 1], ws[:],
                            axis=mybir.AxisListType.X, op=mybir.AluOpType.add,
                            apply_absolute_value=True,
                        )
                        transpose_chunk(
                            sn, st[:, :, ot * P:(ot + 1) * P], c * ssub
                        )
                    redt = small.tile([P, 1], F32, tag="redt")
                    nc.vector.tensor_reduce(
                        redt[:], red[:],
                        axis=mybir.AxisListType.X, op=mybir.AluOpType.add,
                    )
                    nc.vector.tensor_scalar(
                        scale_cols[:, ot:ot + 1], redt[:],
                        1.0 / in_f, 1e-6,
                        op0=mybir.AluOpType.mult, op1=mybir.AluOpType.max,
                    )
                nc.gpsimd.dma_start(
                    scale_pm[:, oc * o_tiles:(oc + 1) * o_tiles], scale_cols[:]
                )
                sc_bc = scbc.tile([P, och], F32, tag="scbc")
                sc_slice = scale_dram[oc * och:(oc + 1) * och]
                nc.sync.dma_start(
                    sc_bc[:],
                    bass.AP(tensor=sc_slice.tensor, offset=sc_slice.offset,
                            ap=[[0, P]] + list(sc_slice.ap)),
                )
                bias_bc = scbc.tile([P, och], F32, tag="biasbc")
                b_slice = b_dram[oc * och:(oc + 1) * och]
                nc.sync.dma_start(
                    bias_bc[:],
                    bass.AP(tensor=b_slice.tensor, offset=b_slice.offset,
                            ap=[[0, P]] + list(b_slice.ap)),
                )
                return st, sc_bc, bias_bc

            def mm_block(oc, t, st, sc_bc, bias_bc):
                pm = psum_mm.tile([P, och], F32, tag="mmps")
                for i in range(i_tiles):
                    nc.tensor.matmul(
                        pm[:],
                        xt[:, i, t * P:(t + 1) * P],
                        st[:, i, :],
                        start=(i == 0), stop=(i == i_tiles - 1),
                    )
                ob = outsb.tile([P, och], F32, tag="ob")
                nc.vector.tensor_mul(out=ob[:], in0=pm[:], in1=sc_bc[:])
                nc.vector.tensor_add(out=ob[:], in0=ob[:], in1=bias_bc[:])
                nc.sync.dma_start(
                    out_dram[t * P:(t + 1) * P, oc * och:(oc + 1) * och],
                    ob[:],
                )

            for oc in range(oc_chunks):
                ctx = build_st(oc)
                for t in range(t_tiles):
                    mm_block(oc, t, *ctx)

    nc.finalize()
    return nc


_CACHE = {}


def kernel(inputs, weight, bias):
    from concourse.bass_utils import run_bass_kernel_spmd

    x = np.ascontiguousarray(np.asarray(inputs, dtype=np.float32))
    w = np.ascontiguousarray(np.asarray(weight, dtype=np.float32))
    b = np.ascontiguousarray(np.asarray(bias, dtype=np.float32))
    assert x.shape == (TOKENS, IN_FEATURES)
    assert w.shape == (OUT_FEATURES, IN_FEATURES)
    assert b.shape == (OUT_FEATURES,)

    if "nc" not in _CACHE:
        _CACHE["nc"] = build_nc(TOKENS // N_CORES, IN_FEATURES, OUT_FEATURES)
    nc = _CACHE["nc"]

    shards = np.split(x, N_CORES, axis=0)
    in_maps = [{"x": shards[c], "w": w, "b": b} for c in range(N_CORES)]
    trace = bool(os.environ.get("BASS_TRACE"))
    res = run_bass_kernel_spmd(nc, in_maps, list(range(N_CORES)), trace=trace)
    if trace:
        _CACHE["last_result"] = res
        if res.exec_time_ns is not None:
            print(f"HW exec time: {res.exec_time_ns} ns")

    return np.concatenate([res.results[c]["out"] for c in range(N_CORES)], axis=0)
